# revision 34
# baseline (speedup 1.0000x reference)
"""Multi-head attention block (QKV proj -> softmax attention -> out proj ->
residual + LayerNorm) on 8 Trainium2 NeuronCores, data-parallel over batch.

Shapes (hardcoded): B=8, S=1024, H=16, HD=64, D=1024.
Each core runs one batch element.

v4 design:
- Host pre-transposes x (xbT/xcT), pre-casts x + weights to fp8e4m3
  (weights pre-scaled by 32 to dodge fp8 subnormals; descale folded into
  each PSUM->SBUF move), folds bv through wo into bo' = bo + bv @ wo and
  pre-adds the residual bias (xbo = x + bo').
- All four projections run fp8 DoubleRow matmuls (two 128-deep k-tiles
  per instruction = 2x PE throughput); fp8 weights stay resident in SBUF.
- Attention: scoresT per k-tile (bf16 q/k), one batched exp per (h,kt)
  with the key mask as per-partition ACT bias, ctx via fp8 DoubleRow over
  k-tile pairs; a ones column in v' makes ctx-PSUM row 64 the softmax
  denominator.
- Softmax denominators are inverted as exp(-ln(x)) on the ACT engine:
  a DVE reciprocal costs ~5-8 ns per element (microcoded NR) and
  dominated earlier versions; ACT ln+exp is ~1.7 ns/element total.
  All ACT funcs (Exp, Ln, Identity) are pinned to the single
  natural_log_exp_and_others table to avoid per-use table reloads.
- Reps are software-pipelined: projection matmuls of rep r+1 interleave
  into the ACT/DVE-bound attention + epilogue of rep r so the PE stays
  continuously busy (it only reaches its 2.4 GHz p-state in long
  streaks). Emission order defines per-engine program order; Tile's
  semaphores handle cross-engine deps; double-buffered persistent tiles
  decouple adjacent reps.
"""
import os as _os
import sys
import time

sys.path.insert(0, "/opt/trn_rl_repo")

import numpy as np

import concourse.bass as bass
import concourse.bacc as bacc
import concourse.tile as tile
from concourse import mybir
from concourse.bass_utils import run_bass_kernel_spmd
from concourse.hw_specs import get_activation_tables

F32 = mybir.dt.float32
BF16 = mybir.dt.bfloat16
FP8 = mybir.dt.float8e4
AF = mybir.ActivationFunctionType
DR = mybir.MatmulPerfMode.DoubleRow
MUL = mybir.AluOpType.mult
ADD = mybir.AluOpType.add
SUB = mybir.AluOpType.subtract

B, S, H, HD = 8, 1024, 16, 64
D = H * HD
NINF = -10000.0
EPS = 1e-6
ST = S // 128    # 8 s-tiles
DT = D // 128    # 8 d-tiles
NH = S // 512    # 2 q halves
WS = 32.0        # host weight pre-scale
CS = 2.0         # extra ctx scale (ctxT = 64*ctx in fp8)

USE_BCAST = _os.environ.get("K_BCAST", "1") == "1"  # 1: Pool broadcast;
# 0: PE-matmul broadcast (Pool ops cost ~2.4us + ~4us semaphores each)
USE_DP = _os.environ.get("K_DP", "1") == "1"  # DoublePixel fp8 scores


def _pin_act_tables(arch):
    """Restrict Exp/Ln/Identity to the one table containing all three, so
    the kernel never reloads activation tables. Mutates the cached dict;
    indices into act_info.json are unchanged."""
    tabs = get_activation_tables(arch)
    keep = "natural_log_exp_and_others"
    if keep not in tabs:
        return
    for name, s in tabs.items():
        if name != keep:
            for f in (AF.Exp, AF.Ln, AF.Identity):
                s.discard(f)


def _bc64(ap_row):
    """View a [1, N] slice as [64, N] via a stride-0 partition dim."""
    return bass.AP(tensor=ap_row.tensor, offset=ap_row.offset,
                   ap=[[0, 64]] + [list(d) for d in ap_row.ap[1:]])


class Ctx:
    """Per-build state: nc, pools, const tiles, weight tiles."""


def _h1_steps(nc, X, io, kc):
    """Projection half of a rep: DMA xT, then Q/K/V matmul groups."""
    xbo, xbT8, xcT8, maskneg, out = io
    SC = kc * 128
    steps = []
    t = {}

    def s_dma():
        t["xT8"] = X.xTp.tile([128, DT, S], FP8, tag="xT8", name="xT8")
        nc.sync.dma_start(
            out=t["xT8"], in_=xbT8.rearrange("(t p) n -> p t n", p=128))
        t["xTc8"] = X.xTp.tile([128, DT, SC], FP8, tag="xTc8", name="xTc8")
        nc.sync.dma_start(
            out=t["xTc8"], in_=xcT8.rearrange("(t p) n -> p t n", p=128))
        t["qT"] = X.persist.tile([128, DT, S],
                         FP8 if USE_DP else BF16,
                         tag="qT", name="qT")
        t["kT"] = X.persist.tile([128, DT, SC],
                         FP8 if USE_DP else BF16,
                         tag="kT", name="kT")
        t["vp"] = X.persist.tile([128, kc, H * (HD + 1)], FP8, tag="vp",
                                 name="vp")
        t["vp_h"] = t["vp"].rearrange("p t (h j) -> p t h j", j=HD + 1)
        nc.vector.memset(t["vp_h"][:, :, :, HD], 1.0)
    steps.append(s_dma)

    # Q: psum = (32 wq)^T x ; DVE writes qT = psum/32 + bq (bf16)
    for mg in range(DT):
        for qh in range(NH):
            def s_q(mg=mg, qh=qh):
                qps = X.ps1.tile([128, 512], F32, tag="ppsum", name="qps")
                for dp in range(DT // 2):
                    nc.tensor.matmul(
                        qps,
                        X.wq_sb[:, 2 * dp:2 * dp + 2,
                                mg * 128:(mg + 1) * 128],
                        t["xT8"][:, 2 * dp:2 * dp + 2,
                                 qh * 512:(qh + 1) * 512],
                        start=(dp == 0), stop=(dp == DT // 2 - 1),
                        perf_mode=DR)
                nc.vector.tensor_scalar(
                    t["qT"][:, mg, qh * 512:(qh + 1) * 512], qps,
                    1.0 / WS, X.bq_sb[:, mg:mg + 1], op0=MUL, op1=ADD)
            steps.append(s_q)

    # K over compacted keys
    kn = SC // 2
    for mg in range(DT):
        for half in range(2):
            def s_k(mg=mg, half=half):
                n0 = half * kn
                kps = X.ps1.tile([128, 512], F32, tag="ppsum", name="kps")
                for dp in range(DT // 2):
                    nc.tensor.matmul(
                        kps[:, 0:kn],
                        X.wk_sb[:, 2 * dp:2 * dp + 2,
                                mg * 128:(mg + 1) * 128],
                        t["xTc8"][:, 2 * dp:2 * dp + 2, n0:n0 + kn],
                        start=(dp == 0), stop=(dp == DT // 2 - 1),
                        perf_mode=DR)
                nc.vector.tensor_scalar(
                    t["kT"][:, mg, n0:n0 + kn], kps[:, 0:kn],
                    1.0 / WS, X.bk_sb[:, mg:mg + 1], op0=MUL, op1=ADD)
            steps.append(s_k)

    # V natural: vp holds 32*v (fp8); ones column pre-set by memset
    for et in range(2):
        for st in range(kc):
            def s_v(et=et, st=st):
                vps = X.ps1.tile([128, 512], F32, tag="ppsum", name="vps")
                for dp in range(DT // 2):
                    nc.tensor.matmul(
                        vps,
                        t["xTc8"][:, 2 * dp:2 * dp + 2,
                                  st * 128:(st + 1) * 128],
                        X.wv_sb[:, 2 * dp:2 * dp + 2,
                                et * 512:(et + 1) * 512],
                        start=(dp == 0), stop=(dp == DT // 2 - 1),
                        perf_mode=DR)
                nc.vector.tensor_copy(
                    t["vp_h"][:, st, 8 * et:8 * et + 8, 0:HD],
                    vps.rearrange("p (h j) -> p h j", j=HD))
            steps.append(s_v)

    return steps, t


def _h2_steps(nc, X, io, t, kc):
    """Attention + out-proj + epilogue of one rep, as emission steps."""
    xbo, xbT8, xcT8, maskneg, out = io
    NP = (kc + 1) // 2
    steps = []

    qT, kT, vp_h = t["qT"], t["kT"], t["vp_h"]
    ctxTs = [X.ctxTp.tile([128, DT, 512], FP8, tag=f"ctxT{qh}", name="ctxT")
             for qh in range(NH)]
    cps_by_head = {}

    for h in range(H):
        base = (h % 2) * 64
        dt = h // 2

        for kp in range(NP):
            def s_kp(h=h, kp=kp, base=base, dt=dt):
                if kp == 0:
                    cps_by_head[h] = [
                        X.ctxps.tile([HD + 1, 512], F32, tag=f"ctx{qh}",
                                     name="ctx_ps") for qh in range(NH)]
                cps = cps_by_head[h]
                kts = [kt_ for kt_ in (2 * kp, 2 * kp + 1) if kt_ < kc]
                e2 = X.Ep.tile([128, 2, NH, 512], FP8, tag="E", name="e2")
                for i, kt_ in enumerate(kts):
                    scp = X.scps.tile([128, NH, 512], F32, tag="scp",
                                      name="scp")
                    for qh in range(NH):
                        nc.tensor.matmul(
                            scp[:, qh, :],
                            kT[base:base + 64, dt,
                               kt_ * 128:(kt_ + 1) * 128],
                            qT[base:base + 64, dt,
                               qh * 512:(qh + 1) * 512],
                            start=True, stop=True,
                            perf_mode=(mybir.MatmulPerfMode.DoublePixel
                                       if USE_DP else None))
                    nc.scalar.activation(
                        e2[:, i], scp, AF.Exp,
                        bias=X.mneg_sb[:, kt_:kt_ + 1], scale=0.125)
                for qh in range(NH):
                    if len(kts) == 2:
                        nc.tensor.matmul(
                            cps[qh],
                            vp_h[:, 2 * kp:2 * kp + 2, h, :],
                            e2[:, :, qh, :],
                            start=(kp == 0), stop=(kp == NP - 1),
                            perf_mode=DR)
                    else:
                        nc.tensor.matmul(
                            cps[qh],
                            vp_h[:, kts[0], h, :],
                            e2[:, 0, qh, :],
                            start=(kp == 0), stop=(kp == NP - 1))
            steps.append(s_kp)

        def s_norm(h=h, base=base, dt=dt):
            cps = cps_by_head[h]
            # invert denominators as exp(-ln(x)) on ACT, reading the
            # colsum rows straight out of the ctx PSUM tiles
            rinv = X.nrm.tile([1, NH * 512],
                              F32 if USE_BCAST else BF16,
                              tag="rinv", name="rinv")
            for qh in range(NH):
                nc.scalar.activation(
                    rinv[0:1, qh * 512:(qh + 1) * 512],
                    cps[qh][HD:HD + 1, :], AF.Ln, bias=0.0, scale=1.0)
            nc.scalar.activation(rinv, rinv, AF.Exp, bias=0.0, scale=-1.0)
            if USE_BCAST:
                csb = X.nrm.tile([64, NH * 512], F32, tag="csb", name="csb")
                nc.gpsimd.partition_broadcast(csb, rinv)
                for qh in range(NH):
                    nc.vector.scalar_tensor_tensor(
                        out=ctxTs[qh][base:base + 64, dt, :],
                        in0=cps[qh][0:HD, :], scalar=CS,
                        in1=csb[:, qh * 512:(qh + 1) * 512],
                        op0=MUL, op1=MUL)
            else:
                # broadcast on the PE: ones64^T(1x64) @ rinv-row -> [64,512]
                for qh in range(NH):
                    bcp = X.ps1.tile([128, 512], F32, tag="ppsum",
                                     name="bcp")
                    nc.tensor.matmul(
                        bcp[0:64, :], X.ones64,
                        rinv[0:1, qh * 512:(qh + 1) * 512],
                        start=True, stop=True)
                    nc.vector.scalar_tensor_tensor(
                        out=ctxTs[qh][base:base + 64, dt, :],
                        in0=cps[qh][0:HD, :], scalar=CS,
                        in1=bcp[0:64, :],
                        op0=MUL, op1=MUL)
        steps.append(s_norm)

    # out projection + epilogue
    for qh in range(NH):
        for qt in range(4):
            def s_oproj(qh=qh, qt=qt):
                stg = qh * 4 + qt
                ctxT = ctxTs[qh]
                pp = []
                for et in range(2):
                    pps = X.ps1.tile([128, 512], F32, tag="ppsum",
                                     name="pps")
                    for dp in range(DT // 2):
                        nc.tensor.matmul(
                            pps,
                            ctxT[:, 2 * dp:2 * dp + 2,
                                 qt * 128:(qt + 1) * 128],
                            X.wo_sb[:, 2 * dp:2 * dp + 2,
                                    et * 512:(et + 1) * 512],
                            start=(dp == 0), stop=(dp == DT // 2 - 1),
                            perf_mode=DR)
                    pp.append(pps)
                x_t = X.xep.tile([128, D], F32, tag="xe", name="x_e")
                nc.sync.dma_start(
                    out=x_t, in_=xbo[stg * 128:(stg + 1) * 128, :])
                t_ = X.epi.tile([128, D], F32, tag="t", name="t")
                for et in range(2):
                    # t = proj/2048 + (x + bo')  — residual fused w/ descale
                    nc.vector.scalar_tensor_tensor(
                        out=t_[:, et * 512:(et + 1) * 512],
                        in0=pp[et], scalar=1.0 / (WS * WS * CS),
                        in1=x_t[:, et * 512:(et + 1) * 512],
                        op0=MUL, op1=ADD)
                stats = X.epi.tile([128, 2, nc.vector.BN_STATS_DIM], F32,
                                   tag="stats", name="stats")
                tg = t_.rearrange("p (g d) -> p g d", g=2)
                for g in range(2):
                    nc.vector.bn_stats(stats[:, g, :], tg[:, g, :])
                mv = X.epi.tile([128, nc.vector.BN_AGGR_DIM], F32, tag="mv",
                                name="mv")
                nc.vector.bn_aggr(mv, stats)
                # rstd = exp(-0.5*ln(var+eps)); same act table as Exp
                lnv = X.epi.tile([128, 1], F32, tag="lnv", name="lnv")
                nc.scalar.activation(
                    lnv, mv[:, 1:2], AF.Ln, bias=X.eps_sb, scale=1.0)
                rstd = X.epi.tile([128, 1], F32, tag="rstd", name="rstd")
                nc.scalar.activation(rstd, lnv, AF.Exp, bias=0.0, scale=-0.5)
                nc.vector.tensor_scalar(
                    t_, t_, mv[:, 0:1], rstd, op0=SUB, op1=MUL)
                nc.gpsimd.tensor_mul(t_, t_, X.gamma_b)
                nc.gpsimd.tensor_add(t_, t_, X.beta_b)
                nc.sync.dma_start(
                    out=out[stg * 128:(stg + 1) * 128, :], in_=t_)
            steps.append(s_oproj)

    return steps


def build_bass(reps=1, kc=8):
    nc = bacc.Bacc("TRN2", target_bir_lowering=False, debug=False)
    if _os.environ.get("K_PIN", "1") == "1":
        _pin_act_tables(nc.m.arch)

    SC = kc * 128
    xbo = nc.dram_tensor("xbo", [S, D], F32, kind="ExternalInput").ap()
    xbT8 = nc.dram_tensor("xbT8", [D, S], FP8, kind="ExternalInput").ap()
    xcT8 = nc.dram_tensor("xcT8", [D, SC], FP8, kind="ExternalInput").ap()
    wq8 = nc.dram_tensor("wq8", [D, D], FP8, kind="ExternalInput").ap()
    wk8 = nc.dram_tensor("wk8", [D, D], FP8, kind="ExternalInput").ap()
    wv8 = nc.dram_tensor("wv8", [D, D], FP8, kind="ExternalInput").ap()
    wo8 = nc.dram_tensor("wo8", [D, D], FP8, kind="ExternalInput").ap()
    bq = nc.dram_tensor("bq", [D], F32, kind="ExternalInput").ap()
    bk = nc.dram_tensor("bk", [D], F32, kind="ExternalInput").ap()
    maskneg = nc.dram_tensor("maskneg", [SC], F32, kind="ExternalInput").ap()
    gamma = nc.dram_tensor("gamma", [D], F32, kind="ExternalInput").ap()
    beta = nc.dram_tensor("beta", [D], F32, kind="ExternalInput").ap()
    out = nc.dram_tensor("out", [S, D], F32, kind="ExternalOutput").ap()
    io = (xbo, xbT8, xcT8, maskneg, out)

    X = Ctx()
    with tile.TileContext(nc) as tc:
        with (
            tc.tile_pool(name="const", bufs=1) as const,
            tc.tile_pool(name="wres", bufs=1) as wres,
            tc.tile_pool(name="persist", bufs=2) as persist,
            tc.tile_pool(name="xTp", bufs=1) as xTp,
            tc.tile_pool(name="ctxTp", bufs=2) as ctxTp,
            tc.tile_pool(name="Ep", bufs=4) as Ep,
            tc.tile_pool(name="nrm", bufs=3) as nrm,
            tc.tile_pool(name="xep", bufs=3) as xep,
            tc.tile_pool(name="epi", bufs=3) as epi,
            tc.tile_pool(name="ps1", bufs=2, space="PSUM") as ps1,
            tc.tile_pool(name="scps", bufs=2, space="PSUM") as scps,
            tc.tile_pool(name="ctxps", bufs=1, space="PSUM") as ctxps,
        ):
            X.persist, X.xTp, X.ctxTp, X.Ep, X.nrm = (persist, xTp, ctxTp,
                                                      Ep, nrm)
            X.xep, X.epi, X.ps1, X.scps, X.ctxps = xep, epi, ps1, scps, ctxps

            X.bq_sb = const.tile([128, DT], F32, name="bq_sb")
            nc.sync.dma_start(out=X.bq_sb,
                              in_=bq.rearrange("(t p) -> p t", p=128))
            X.bk_sb = const.tile([128, DT], F32, name="bk_sb")
            nc.sync.dma_start(out=X.bk_sb,
                              in_=bk.rearrange("(t p) -> p t", p=128))
            X.mneg_sb = const.tile([128, kc], F32, name="mneg_sb")
            nc.sync.dma_start(out=X.mneg_sb,
                              in_=maskneg.rearrange("(t p) -> p t", p=128))

            def part_bcast(v):
                return bass.AP(tensor=v.tensor, offset=v.offset,
                               ap=[[0, 128]] + list(v.ap))

            X.gamma_b = const.tile([128, D], F32, name="gamma_b")
            nc.gpsimd.dma_start(out=X.gamma_b, in_=part_bcast(gamma))
            X.beta_b = const.tile([128, D], F32, name="beta_b")
            nc.gpsimd.dma_start(out=X.beta_b, in_=part_bcast(beta))
            X.eps_sb = const.tile([128, 1], F32, name="eps_sb")
            nc.vector.memset(X.eps_sb, EPS)
            X.ones64 = const.tile([1, 64], BF16, name="ones64")
            nc.vector.memset(X.ones64, 1.0)

            # resident fp8 weights
            for nm, w in (("wq_sb", wq8), ("wk_sb", wk8), ("wv_sb", wv8),
                          ("wo_sb", wo8)):
                tl = wres.tile([128, DT, D], FP8, name=nm)
                nc.sync.dma_start(
                    out=tl, in_=w.rearrange("(t p) n -> p t n", p=128))
                setattr(X, nm, tl)

            # software-pipelined reps: H1(r+1) interleaves into H2(r)
            h1, t_cur = _h1_steps(nc, X, io, kc)
            for s in h1:
                s()
            for r in range(reps):
                h2 = _h2_steps(nc, X, io, t_cur, kc)
                if r + 1 < reps:
                    h1n, t_nxt = _h1_steps(nc, X, io, kc)
                else:
                    h1n, t_nxt = [], None
                inj = 0
                for s in h2:
                    s()
                    if inj < len(h1n):
                        h1n[inj]()
                        inj += 1
                while inj < len(h1n):
                    h1n[inj]()
                    inj += 1
                t_cur = t_nxt

    nc.compile()
    return nc


_NC_CACHE = {}


def _get_nc(reps=1, kc=8):
    if (reps, kc) not in _NC_CACHE:
        _NC_CACHE[(reps, kc)] = build_bass(reps, kc)
    return _NC_CACHE[(reps, kc)]


def make_in_maps(x, mask, wq, bq, wk, bk, wv, bv, wo, bo, gamma, beta):
    f8 = mybir.dt.np(FP8)
    x = np.asarray(x, dtype=np.float32)
    mask = np.asarray(mask)
    maskneg = (mask.astype(np.float32) * NINF).astype(np.float32)
    # host-side k-compaction: unmasked rows first, masked filler after
    n_un = int((mask == 0).sum(axis=1).max())
    kc = min(max((n_un + 127) // 128, 1), ST)
    SC = kc * 128
    idxs = [np.argsort(mask[c], kind="stable")[:SC] for c in range(B)]
    wq = np.asarray(wq, np.float32)
    wk = np.asarray(wk, np.float32)
    wv = np.asarray(wv, np.float32)
    wo = np.asarray(wo, np.float32)
    bv = np.asarray(bv, np.float32)
    bo = np.asarray(bo, np.float32)
    bop = bo + bv @ wo   # fold v-bias through the output projection
    common = {
        "wq8": (wq * WS).astype(f8), "wk8": (wk * WS).astype(f8),
        "wv8": (wv * WS).astype(f8), "wo8": (wo * WS).astype(f8),
        "bq": np.asarray(bq, np.float32), "bk": np.asarray(bk, np.float32),
        "gamma": np.asarray(gamma, np.float32),
        "beta": np.asarray(beta, np.float32),
    }
    maps = []
    for c in range(B):
        xc = x[c][idxs[c]]
        maps.append(dict(
            common,
            xbo=np.ascontiguousarray(x[c] + bop[None, :]),
            xbT8=np.ascontiguousarray(x[c].T).astype(f8),
            xcT8=np.ascontiguousarray(xc.T).astype(f8),
            maskneg=np.ascontiguousarray(maskneg[c][idxs[c]])))
    return maps, kc


def kernel(x, mask, wq, bq, wk, bk, wv, bv, wo, bo, gamma, beta):
    in_maps, kc = make_in_maps(x, mask, wq, bq, wk, bk, wv, bv, wo, bo,
                               gamma, beta)
    nc = _get_nc(1, kc)
    last_err = None
    for _ in range(3):
        try:
            res = run_bass_kernel_spmd(nc, in_maps, core_ids=list(range(B)))
            return np.stack([res.results[c]["out"] for c in range(B)], axis=0)
        except Exception as e:  # transient NRT device errors: retry
            last_err = e
            time.sleep(5)
    raise last_err


# revision 35
# speedup vs baseline: 1.0037x; 1.0037x over previous
"""Multi-head attention block (QKV proj -> softmax attention -> out proj ->
residual + LayerNorm) on 8 Trainium2 NeuronCores, data-parallel over batch.

Shapes (hardcoded): B=8, S=1024, H=16, HD=64, D=1024.
Each core runs one batch element.

v4 design:
- Host pre-transposes x (xbT/xcT), pre-casts x + weights to fp8e4m3
  (weights pre-scaled by 32 to dodge fp8 subnormals; descale folded into
  each PSUM->SBUF move), folds bv through wo into bo' = bo + bv @ wo and
  pre-adds the residual bias (xbo = x + bo').
- All four projections run fp8 DoubleRow matmuls (two 128-deep k-tiles
  per instruction = 2x PE throughput); fp8 weights stay resident in SBUF.
- Attention: scoresT per k-tile (bf16 q/k), one batched exp per (h,kt)
  with the key mask as per-partition ACT bias, ctx via fp8 DoubleRow over
  k-tile pairs; a ones column in v' makes ctx-PSUM row 64 the softmax
  denominator.
- Softmax denominators are inverted as exp(-ln(x)) on the ACT engine:
  a DVE reciprocal costs ~5-8 ns per element (microcoded NR) and
  dominated earlier versions; ACT ln+exp is ~1.7 ns/element total.
  All ACT funcs (Exp, Ln, Identity) are pinned to the single
  natural_log_exp_and_others table to avoid per-use table reloads.
- Reps are software-pipelined: projection matmuls of rep r+1 interleave
  into the ACT/DVE-bound attention + epilogue of rep r so the PE stays
  continuously busy (it only reaches its 2.4 GHz p-state in long
  streaks). Emission order defines per-engine program order; Tile's
  semaphores handle cross-engine deps; double-buffered persistent tiles
  decouple adjacent reps.
"""
import os as _os
import sys
import time

sys.path.insert(0, "/opt/trn_rl_repo")

import numpy as np

import concourse.bass as bass
import concourse.bacc as bacc
import concourse.tile as tile
from concourse import mybir
from concourse.bass_utils import run_bass_kernel_spmd
from concourse.hw_specs import get_activation_tables

F32 = mybir.dt.float32
BF16 = mybir.dt.bfloat16
FP8 = mybir.dt.float8e4
AF = mybir.ActivationFunctionType
DR = mybir.MatmulPerfMode.DoubleRow
MUL = mybir.AluOpType.mult
ADD = mybir.AluOpType.add
SUB = mybir.AluOpType.subtract

B, S, H, HD = 8, 1024, 16, 64
D = H * HD
NINF = -10000.0
EPS = 1e-6
ST = S // 128    # 8 s-tiles
DT = D // 128    # 8 d-tiles
NH = S // 512    # 2 q halves
WS = 32.0        # host weight pre-scale
CS = 2.0         # extra ctx scale (ctxT = 64*ctx in fp8)

USE_BCAST = _os.environ.get("K_BCAST", "1") == "1"  # 1: Pool broadcast;
# 0: PE-matmul broadcast (Pool ops cost ~2.4us + ~4us semaphores each)
USE_DP = _os.environ.get("K_DP", "0") == "1"  # DoublePixel fp8 scores


def _pin_act_tables(arch):
    """Restrict Exp/Ln/Identity to the one table containing all three, so
    the kernel never reloads activation tables. Mutates the cached dict;
    indices into act_info.json are unchanged."""
    tabs = get_activation_tables(arch)
    keep = "natural_log_exp_and_others"
    if keep not in tabs:
        return
    for name, s in tabs.items():
        if name != keep:
            for f in (AF.Exp, AF.Ln, AF.Identity):
                s.discard(f)


def _bc64(ap_row):
    """View a [1, N] slice as [64, N] via a stride-0 partition dim."""
    return bass.AP(tensor=ap_row.tensor, offset=ap_row.offset,
                   ap=[[0, 64]] + [list(d) for d in ap_row.ap[1:]])


class Ctx:
    """Per-build state: nc, pools, const tiles, weight tiles."""


def _h1_steps(nc, X, io, kc):
    """Projection half of a rep: DMA xT, then Q/K/V matmul groups."""
    xbo, xbT8, xcT8, maskneg, out = io
    SC = kc * 128
    steps = []
    t = {}

    def s_dma():
        t["xT8"] = X.xTp.tile([128, DT, S], FP8, tag="xT8", name="xT8")
        nc.sync.dma_start(
            out=t["xT8"], in_=xbT8.rearrange("(t p) n -> p t n", p=128))
        t["xTc8"] = X.xTp.tile([128, DT, SC], FP8, tag="xTc8", name="xTc8")
        nc.sync.dma_start(
            out=t["xTc8"], in_=xcT8.rearrange("(t p) n -> p t n", p=128))
        t["qT"] = X.persist.tile([128, DT, S],
                         FP8 if USE_DP else BF16,
                         tag="qT", name="qT")
        t["kT"] = X.persist.tile([128, DT, SC],
                         FP8 if USE_DP else BF16,
                         tag="kT", name="kT")
        t["vp"] = X.persist.tile([128, kc, H * (HD + 1)], FP8, tag="vp",
                                 name="vp")
        t["vp_h"] = t["vp"].rearrange("p t (h j) -> p t h j", j=HD + 1)
        nc.vector.memset(t["vp_h"][:, :, :, HD], 1.0)
    steps.append(s_dma)

    # Q: psum = (32 wq)^T x ; DVE writes qT = psum/32 + bq (bf16)
    for mg in range(DT):
        for qh in range(NH):
            def s_q(mg=mg, qh=qh):
                qps = X.ps1.tile([128, 512], F32, tag="ppsum", name="qps")
                for dp in range(DT // 2):
                    nc.tensor.matmul(
                        qps,
                        X.wq_sb[:, 2 * dp:2 * dp + 2,
                                mg * 128:(mg + 1) * 128],
                        t["xT8"][:, 2 * dp:2 * dp + 2,
                                 qh * 512:(qh + 1) * 512],
                        start=(dp == 0), stop=(dp == DT // 2 - 1),
                        perf_mode=DR)
                nc.vector.tensor_scalar(
                    t["qT"][:, mg, qh * 512:(qh + 1) * 512], qps,
                    1.0 / WS, X.bq_sb[:, mg:mg + 1], op0=MUL, op1=ADD)
            steps.append(s_q)

    # K over compacted keys
    kn = SC // 2
    for mg in range(DT):
        for half in range(2):
            def s_k(mg=mg, half=half):
                n0 = half * kn
                kps = X.ps1.tile([128, 512], F32, tag="ppsum", name="kps")
                for dp in range(DT // 2):
                    nc.tensor.matmul(
                        kps[:, 0:kn],
                        X.wk_sb[:, 2 * dp:2 * dp + 2,
                                mg * 128:(mg + 1) * 128],
                        t["xTc8"][:, 2 * dp:2 * dp + 2, n0:n0 + kn],
                        start=(dp == 0), stop=(dp == DT // 2 - 1),
                        perf_mode=DR)
                nc.vector.tensor_scalar(
                    t["kT"][:, mg, n0:n0 + kn], kps[:, 0:kn],
                    1.0 / WS, X.bk_sb[:, mg:mg + 1], op0=MUL, op1=ADD)
            steps.append(s_k)

    # V natural: vp holds 32*v (fp8); ones column pre-set by memset
    for et in range(2):
        for st in range(kc):
            def s_v(et=et, st=st):
                vps = X.ps1.tile([128, 512], F32, tag="ppsum", name="vps")
                for dp in range(DT // 2):
                    nc.tensor.matmul(
                        vps,
                        t["xTc8"][:, 2 * dp:2 * dp + 2,
                                  st * 128:(st + 1) * 128],
                        X.wv_sb[:, 2 * dp:2 * dp + 2,
                                et * 512:(et + 1) * 512],
                        start=(dp == 0), stop=(dp == DT // 2 - 1),
                        perf_mode=DR)
                nc.vector.tensor_copy(
                    t["vp_h"][:, st, 8 * et:8 * et + 8, 0:HD],
                    vps.rearrange("p (h j) -> p h j", j=HD))
            steps.append(s_v)

    return steps, t


def _h2_steps(nc, X, io, t, kc):
    """Attention + out-proj + epilogue of one rep, as emission steps."""
    xbo, xbT8, xcT8, maskneg, out = io
    NP = (kc + 1) // 2
    steps = []

    qT, kT, vp_h = t["qT"], t["kT"], t["vp_h"]
    ctxTs = [X.ctxTp.tile([128, DT, 512], FP8, tag=f"ctxT{qh}", name="ctxT")
             for qh in range(NH)]
    cps_by_head = {}

    for h in range(H):
        base = (h % 2) * 64
        dt = h // 2

        for kp in range(NP):
            def s_kp(h=h, kp=kp, base=base, dt=dt):
                if kp == 0:
                    cps_by_head[h] = [
                        X.ctxps.tile([HD + 1, 512], F32, tag=f"ctx{qh}",
                                     name="ctx_ps") for qh in range(NH)]
                cps = cps_by_head[h]
                kts = [kt_ for kt_ in (2 * kp, 2 * kp + 1) if kt_ < kc]
                e2 = X.Ep.tile([128, 2, NH, 512], FP8, tag="E", name="e2")
                for i, kt_ in enumerate(kts):
                    scp = X.scps.tile([128, NH, 512], F32, tag="scp",
                                      name="scp")
                    for qh in range(NH):
                        nc.tensor.matmul(
                            scp[:, qh, :],
                            kT[base:base + 64, dt,
                               kt_ * 128:(kt_ + 1) * 128],
                            qT[base:base + 64, dt,
                               qh * 512:(qh + 1) * 512],
                            start=True, stop=True,
                            perf_mode=(mybir.MatmulPerfMode.DoublePixel
                                       if USE_DP else None))
                    nc.scalar.activation(
                        e2[:, i], scp, AF.Exp,
                        bias=X.mneg_sb[:, kt_:kt_ + 1], scale=0.125)
                for qh in range(NH):
                    if len(kts) == 2:
                        nc.tensor.matmul(
                            cps[qh],
                            vp_h[:, 2 * kp:2 * kp + 2, h, :],
                            e2[:, :, qh, :],
                            start=(kp == 0), stop=(kp == NP - 1),
                            perf_mode=DR)
                    else:
                        nc.tensor.matmul(
                            cps[qh],
                            vp_h[:, kts[0], h, :],
                            e2[:, 0, qh, :],
                            start=(kp == 0), stop=(kp == NP - 1))
            steps.append(s_kp)

        def s_norm(h=h, base=base, dt=dt):
            cps = cps_by_head[h]
            # invert denominators as exp(-ln(x)) on ACT, reading the
            # colsum rows straight out of the ctx PSUM tiles
            rinv = X.nrm.tile([1, NH * 512],
                              F32 if USE_BCAST else BF16,
                              tag="rinv", name="rinv")
            for qh in range(NH):
                nc.scalar.activation(
                    rinv[0:1, qh * 512:(qh + 1) * 512],
                    cps[qh][HD:HD + 1, :], AF.Ln, bias=0.0, scale=1.0)
            nc.scalar.activation(rinv, rinv, AF.Exp, bias=0.0, scale=-1.0)
            if USE_BCAST:
                csb = X.nrm.tile([64, NH * 512], F32, tag="csb", name="csb")
                nc.gpsimd.partition_broadcast(csb, rinv)
                for qh in range(NH):
                    nc.vector.scalar_tensor_tensor(
                        out=ctxTs[qh][base:base + 64, dt, :],
                        in0=cps[qh][0:HD, :], scalar=CS,
                        in1=csb[:, qh * 512:(qh + 1) * 512],
                        op0=MUL, op1=MUL)
            else:
                # broadcast on the PE: ones64^T(1x64) @ rinv-row -> [64,512]
                for qh in range(NH):
                    bcp = X.ps1.tile([128, 512], F32, tag="ppsum",
                                     name="bcp")
                    nc.tensor.matmul(
                        bcp[0:64, :], X.ones64,
                        rinv[0:1, qh * 512:(qh + 1) * 512],
                        start=True, stop=True)
                    nc.vector.scalar_tensor_tensor(
                        out=ctxTs[qh][base:base + 64, dt, :],
                        in0=cps[qh][0:HD, :], scalar=CS,
                        in1=bcp[0:64, :],
                        op0=MUL, op1=MUL)
        steps.append(s_norm)

    # out projection + epilogue
    for qh in range(NH):
        for qt in range(4):
            def s_oproj(qh=qh, qt=qt):
                stg = qh * 4 + qt
                ctxT = ctxTs[qh]
                pp = []
                for et in range(2):
                    pps = X.ps1.tile([128, 512], F32, tag="ppsum",
                                     name="pps")
                    for dp in range(DT // 2):
                        nc.tensor.matmul(
                            pps,
                            ctxT[:, 2 * dp:2 * dp + 2,
                                 qt * 128:(qt + 1) * 128],
                            X.wo_sb[:, 2 * dp:2 * dp + 2,
                                    et * 512:(et + 1) * 512],
                            start=(dp == 0), stop=(dp == DT // 2 - 1),
                            perf_mode=DR)
                    pp.append(pps)
                x_t = X.xep.tile([128, D], F32, tag="xe", name="x_e")
                nc.sync.dma_start(
                    out=x_t, in_=xbo[stg * 128:(stg + 1) * 128, :])
                t_ = X.epi.tile([128, D], F32, tag="t", name="t")
                for et in range(2):
                    # t = proj/2048 + (x + bo')  — residual fused w/ descale
                    nc.vector.scalar_tensor_tensor(
                        out=t_[:, et * 512:(et + 1) * 512],
                        in0=pp[et], scalar=1.0 / (WS * WS * CS),
                        in1=x_t[:, et * 512:(et + 1) * 512],
                        op0=MUL, op1=ADD)
                stats = X.epi.tile([128, 2, nc.vector.BN_STATS_DIM], F32,
                                   tag="stats", name="stats")
                tg = t_.rearrange("p (g d) -> p g d", g=2)
                for g in range(2):
                    nc.vector.bn_stats(stats[:, g, :], tg[:, g, :])
                mv = X.epi.tile([128, nc.vector.BN_AGGR_DIM], F32, tag="mv",
                                name="mv")
                nc.vector.bn_aggr(mv, stats)
                # rstd = exp(-0.5*ln(var+eps)); same act table as Exp
                lnv = X.epi.tile([128, 1], F32, tag="lnv", name="lnv")
                nc.scalar.activation(
                    lnv, mv[:, 1:2], AF.Ln, bias=X.eps_sb, scale=1.0)
                rstd = X.epi.tile([128, 1], F32, tag="rstd", name="rstd")
                nc.scalar.activation(rstd, lnv, AF.Exp, bias=0.0, scale=-0.5)
                nc.vector.tensor_scalar(
                    t_, t_, mv[:, 0:1], rstd, op0=SUB, op1=MUL)
                nc.gpsimd.tensor_mul(t_, t_, X.gamma_b)
                nc.gpsimd.tensor_add(t_, t_, X.beta_b)
                nc.sync.dma_start(
                    out=out[stg * 128:(stg + 1) * 128, :], in_=t_)
            steps.append(s_oproj)

    return steps


def build_bass(reps=1, kc=8):
    nc = bacc.Bacc("TRN2", target_bir_lowering=False, debug=False)
    if _os.environ.get("K_PIN", "1") == "1":
        _pin_act_tables(nc.m.arch)

    SC = kc * 128
    xbo = nc.dram_tensor("xbo", [S, D], F32, kind="ExternalInput").ap()
    xbT8 = nc.dram_tensor("xbT8", [D, S], FP8, kind="ExternalInput").ap()
    xcT8 = nc.dram_tensor("xcT8", [D, SC], FP8, kind="ExternalInput").ap()
    wq8 = nc.dram_tensor("wq8", [D, D], FP8, kind="ExternalInput").ap()
    wk8 = nc.dram_tensor("wk8", [D, D], FP8, kind="ExternalInput").ap()
    wv8 = nc.dram_tensor("wv8", [D, D], FP8, kind="ExternalInput").ap()
    wo8 = nc.dram_tensor("wo8", [D, D], FP8, kind="ExternalInput").ap()
    bq = nc.dram_tensor("bq", [D], F32, kind="ExternalInput").ap()
    bk = nc.dram_tensor("bk", [D], F32, kind="ExternalInput").ap()
    maskneg = nc.dram_tensor("maskneg", [SC], F32, kind="ExternalInput").ap()
    gamma = nc.dram_tensor("gamma", [D], F32, kind="ExternalInput").ap()
    beta = nc.dram_tensor("beta", [D], F32, kind="ExternalInput").ap()
    out = nc.dram_tensor("out", [S, D], F32, kind="ExternalOutput").ap()
    io = (xbo, xbT8, xcT8, maskneg, out)

    X = Ctx()
    with tile.TileContext(nc) as tc:
        with (
            tc.tile_pool(name="const", bufs=1) as const,
            tc.tile_pool(name="wres", bufs=1) as wres,
            tc.tile_pool(name="persist", bufs=2) as persist,
            tc.tile_pool(name="xTp", bufs=1) as xTp,
            tc.tile_pool(name="ctxTp", bufs=2) as ctxTp,
            tc.tile_pool(name="Ep", bufs=4) as Ep,
            tc.tile_pool(name="nrm", bufs=3) as nrm,
            tc.tile_pool(name="xep", bufs=3) as xep,
            tc.tile_pool(name="epi", bufs=3) as epi,
            tc.tile_pool(name="ps1", bufs=2, space="PSUM") as ps1,
            tc.tile_pool(name="scps", bufs=2, space="PSUM") as scps,
            tc.tile_pool(name="ctxps", bufs=1, space="PSUM") as ctxps,
        ):
            X.persist, X.xTp, X.ctxTp, X.Ep, X.nrm = (persist, xTp, ctxTp,
                                                      Ep, nrm)
            X.xep, X.epi, X.ps1, X.scps, X.ctxps = xep, epi, ps1, scps, ctxps

            X.bq_sb = const.tile([128, DT], F32, name="bq_sb")
            nc.sync.dma_start(out=X.bq_sb,
                              in_=bq.rearrange("(t p) -> p t", p=128))
            X.bk_sb = const.tile([128, DT], F32, name="bk_sb")
            nc.sync.dma_start(out=X.bk_sb,
                              in_=bk.rearrange("(t p) -> p t", p=128))
            X.mneg_sb = const.tile([128, kc], F32, name="mneg_sb")
            nc.sync.dma_start(out=X.mneg_sb,
                              in_=maskneg.rearrange("(t p) -> p t", p=128))

            def part_bcast(v):
                return bass.AP(tensor=v.tensor, offset=v.offset,
                               ap=[[0, 128]] + list(v.ap))

            X.gamma_b = const.tile([128, D], F32, name="gamma_b")
            nc.gpsimd.dma_start(out=X.gamma_b, in_=part_bcast(gamma))
            X.beta_b = const.tile([128, D], F32, name="beta_b")
            nc.gpsimd.dma_start(out=X.beta_b, in_=part_bcast(beta))
            X.eps_sb = const.tile([128, 1], F32, name="eps_sb")
            nc.vector.memset(X.eps_sb, EPS)
            X.ones64 = const.tile([1, 64], BF16, name="ones64")
            nc.vector.memset(X.ones64, 1.0)

            # resident fp8 weights
            for nm, w in (("wq_sb", wq8), ("wk_sb", wk8), ("wv_sb", wv8),
                          ("wo_sb", wo8)):
                tl = wres.tile([128, DT, D], FP8, name=nm)
                nc.sync.dma_start(
                    out=tl, in_=w.rearrange("(t p) n -> p t n", p=128))
                setattr(X, nm, tl)

            # software-pipelined reps: H1(r+1) interleaves into H2(r)
            h1, t_cur = _h1_steps(nc, X, io, kc)
            for s in h1:
                s()
            for r in range(reps):
                h2 = _h2_steps(nc, X, io, t_cur, kc)
                if r + 1 < reps:
                    h1n, t_nxt = _h1_steps(nc, X, io, kc)
                else:
                    h1n, t_nxt = [], None
                inj = 0
                for s in h2:
                    s()
                    if inj < len(h1n):
                        h1n[inj]()
                        inj += 1
                while inj < len(h1n):
                    h1n[inj]()
                    inj += 1
                t_cur = t_nxt

    nc.compile()
    return nc


_NC_CACHE = {}


def _get_nc(reps=1, kc=8):
    if (reps, kc) not in _NC_CACHE:
        _NC_CACHE[(reps, kc)] = build_bass(reps, kc)
    return _NC_CACHE[(reps, kc)]


def make_in_maps(x, mask, wq, bq, wk, bk, wv, bv, wo, bo, gamma, beta):
    f8 = mybir.dt.np(FP8)
    x = np.asarray(x, dtype=np.float32)
    mask = np.asarray(mask)
    maskneg = (mask.astype(np.float32) * NINF).astype(np.float32)
    # host-side k-compaction: unmasked rows first, masked filler after
    n_un = int((mask == 0).sum(axis=1).max())
    kc = min(max((n_un + 127) // 128, 1), ST)
    SC = kc * 128
    idxs = [np.argsort(mask[c], kind="stable")[:SC] for c in range(B)]
    wq = np.asarray(wq, np.float32)
    wk = np.asarray(wk, np.float32)
    wv = np.asarray(wv, np.float32)
    wo = np.asarray(wo, np.float32)
    bv = np.asarray(bv, np.float32)
    bo = np.asarray(bo, np.float32)
    bop = bo + bv @ wo   # fold v-bias through the output projection
    common = {
        "wq8": (wq * WS).astype(f8), "wk8": (wk * WS).astype(f8),
        "wv8": (wv * WS).astype(f8), "wo8": (wo * WS).astype(f8),
        "bq": np.asarray(bq, np.float32), "bk": np.asarray(bk, np.float32),
        "gamma": np.asarray(gamma, np.float32),
        "beta": np.asarray(beta, np.float32),
    }
    maps = []
    for c in range(B):
        xc = x[c][idxs[c]]
        maps.append(dict(
            common,
            xbo=np.ascontiguousarray(x[c] + bop[None, :]),
            xbT8=np.ascontiguousarray(x[c].T).astype(f8),
            xcT8=np.ascontiguousarray(xc.T).astype(f8),
            maskneg=np.ascontiguousarray(maskneg[c][idxs[c]])))
    return maps, kc


def kernel(x, mask, wq, bq, wk, bk, wv, bv, wo, bo, gamma, beta):
    in_maps, kc = make_in_maps(x, mask, wq, bq, wk, bk, wv, bv, wo, bo,
                               gamma, beta)
    nc = _get_nc(1, kc)
    last_err = None
    for _ in range(3):
        try:
            res = run_bass_kernel_spmd(nc, in_maps, core_ids=list(range(B)))
            return np.stack([res.results[c]["out"] for c in range(B)], axis=0)
        except Exception as e:  # transient NRT device errors: retry
            last_err = e
            time.sleep(5)
    raise last_err
